# revision 31
# baseline (speedup 1.0000x reference)
"""Trainium2 Bass kernel for nn_CrossAttention (dense_transformer).

Sharding: data-parallel over batch B=8 across the 8 NeuronCores (one batch
element per core, conv weights replicated). No collectives.

Per-core dataflow (all matmuls fp16 operands / fp32 PSUM accumulation):
  - depthwise 3x3 convs on VectorE: 9 fused multiply-accumulate taps
    (scalar_tensor_tensor) with edge-clipped rectangles (== SAME padding)
  - pointwise convs on TensorE: Q,K produced in [D,N] layout, V in [N,D]
  - attention computes S^T = K^T-tiles @ Q  (so no transposes anywhere),
    exp on ScalarE (softmax denominator via an all-ones stationary matmul,
    which also broadcasts it across partitions), PV with V as stationary
    operand -> output directly in [D,N] layout
  - normalize+ReLU fused into one scalar_tensor_tensor: max(pv,0)*inv(den)
"""

import numpy as np

import concourse.bacc as bacc
import concourse.tile as tile
from concourse import mybir
from concourse import bass_utils

B, C, H, W, D = 8, 256, 48, 48, 256
N = H * W          # 2304 tokens
P = 128            # partitions
CT = C // P        # 2 channel tiles
NT = N // P        # 18 token tiles
CW = 384           # attention chunk width (tokens)
NCHUNK = N // CW   # 6 chunks
RPC = H // NCHUNK  # 8 rows per chunk

f16 = mybir.dt.float16
f32 = mybir.dt.float32
Alu = mybir.AluOpType
Act = mybir.ActivationFunctionType


def _build_nc():
    nc = bacc.Bacc("TRN2", target_bir_lowering=False, debug=False, num_devices=8)

    dram = {}
    def din(name, shape, dt):
        dram[name] = nc.dram_tensor(name, shape, dt, kind="ExternalInput").ap()
    din("x", [C, N], f16)
    din("y", [C, N], f16)
    for br in ("q", "v"):
        din(br + "dw", [P, CT, 9], f32)    # depthwise weights, per-channel
        din(br + "db", [P, CT], f32)       # depthwise bias
        din(br + "pwT", [P, CT, D], f16)   # pointwise weights, [c_in, d] layout
    # K branch is a folded full 3x3 conv on TensorE: W2[c,tap,d] (x64 scaled)
    din("kw2", [P, CT, 9, D], f16)
    din("kpb2", [P, 2], f32)               # folded K bias (per-d)
    din("qpb", [P, 2], f32)                # pointwise bias for Q (per-d)
    din("vpb", [1, D], f16)                # pointwise bias for V (bias row)
    out_ap = nc.dram_tensor("out", [C, N], f32, kind="ExternalOutput").ap()

    with tile.TileContext(nc) as tc:
        with (
            tc.tile_pool(name="persist", bufs=1) as persist,
            tc.tile_pool(name="ps", bufs=2, space="PSUM") as ps_pool,
            tc.tile_pool(name="ost", bufs=4) as ost,
            tc.tile_pool(name="dwtmp", bufs=8) as dwtmp,
        ):
            # ---- inputs to SBUF first (they gate everything) ----
            # y goes into a zero-padded [50,50] image for the folded K conv;
            # only the border cells need zeroing (cheap strided memsets)
            y_pad = persist.tile([P, CT, H + 2, W + 2], f16, tag="y_pad")
            for ct in range(CT):
                nc.vector.memset(y_pad[:, ct, 0, :], 0.0)
                nc.vector.memset(y_pad[:, ct, H + 1, :], 0.0)
                nc.vector.memset(y_pad[:, ct, :, 0], 0.0)
                nc.vector.memset(y_pad[:, ct, :, W + 1], 0.0)
            x_sb = persist.tile([P, CT, N], f16, tag="x_sb")

            wsb = {}
            def wtile(name):
                t = persist.tile(list(dram[name].shape), dram[name].dtype,
                                 tag=name, name=name)
                wsb[name] = t
                return t

            # critical-path tensors on the fast HWDGE queue, in need order:
            # folded-K needs y_pad + kw2 first, VectorE needs x + qdw next
            t = wtile("kw2")
            for ct in range(CT):
                nc.sync.dma_start(
                    y_pad[:, ct, 1:H + 1, 1:W + 1],
                    dram["y"][ct * P:(ct + 1) * P, :].rearrange(
                        "p (h w) -> p h w", w=W))
                nc.sync.dma_start(t[:, ct], dram["kw2"][:, ct])
            for ct in range(CT):
                nc.sync.dma_start(x_sb[:, ct, :], dram["x"][ct * P:(ct + 1) * P, :])

            # remaining weights on the gpsimd SWDGE queue
            for name in ("qdw", "qdb", "kpb2", "vdw", "vdb",
                         "qpwT", "vpwT", "qpb", "vpb"):
                nc.gpsimd.dma_start(wtile(name), dram[name])
            ones128 = persist.tile([P, P], f16, tag="ones128")
            nc.vector.memset(ones128, 1.0)
            ones1 = persist.tile([1, P], f16, tag="ones1")
            nc.vector.memset(ones1, 1.0)

            # ---- persistent intermediates ----
            xq_sb = persist.tile([P, CT, N], f16, tag="xq_sb")  # dw(x) for Q
            xv_sb = persist.tile([P, CT, N], f16, tag="xv_sb")  # dw(x) for V
            Q_sb = persist.tile([P, 2, N], f16, tag="Q_sb")     # [d, n]
            K_sb = persist.tile([P, 2, N], f16, tag="K_sb")     # [d, n]
            V_sb = persist.tile([P, NT, D], f16, tag="V_sb")    # [n, d]
            pT_all = persist.tile([P, NCHUNK, NT, CW], f16, tag="pT")

            # taps computed on ScalarE (idle early) for the K/V branches
            ACT_TAPS = (0, 2, 6, 8)

            def dw_rows(dst, src, w_sb, b_sb, r0, r1, use_act=False,
                        use_pool=False):
                """Depthwise 3x3 (SAME) on out rows [r0, r1).

                tensor_scalar products (4x DVE mode) + tensor_tensor adds (2x);
                optionally 4 of the 8 edge-tap products go to ScalarE.
                """
                for ct in range(CT):
                    a3 = dst[:, ct, :].rearrange("p (h w) -> p h w", w=W)
                    x3 = src[:, ct, :].rearrange("p (h w) -> p h w", w=W)
                    # center tap initializes the accumulator (+ bias)
                    nc.vector.tensor_scalar(
                        a3[:, r0:r1, :], x3[:, r0:r1, :],
                        w_sb[:, ct, 4:5], b_sb[:, ct:ct + 1],
                        op0=Alu.mult, op1=Alu.add)
                    for kh in range(3):
                        for kw in range(3):
                            if kh == 1 and kw == 1:
                                continue
                            dy, dx = kh - 1, kw - 1
                            h0 = max(r0, -dy, 0)
                            h1 = min(r1, H - max(0, dy))
                            w0 = max(0, -dx)
                            w1 = W - max(0, dx)
                            if h0 >= h1:
                                continue
                            t = kh * 3 + kw
                            prod = dwtmp.tile([P, H, W], f16, tag="prod",
                                              name="prod")
                            src_rect = x3[:, h0 + dy:h1 + dy, w0 + dx:w1 + dx]
                            p_rect = prod[:, h0:h1, w0:w1]
                            if use_pool and t in ACT_TAPS:
                                nc.gpsimd.tensor_scalar(
                                    p_rect, src_rect, w_sb[:, ct, t:t + 1],
                                    None, op0=Alu.mult)
                            elif use_act and t in ACT_TAPS:
                                nc.scalar.activation(
                                    out=p_rect, in_=src_rect, func=Act.Copy,
                                    scale=w_sb[:, ct, t:t + 1])
                            else:
                                nc.vector.tensor_scalar(
                                    p_rect, src_rect, w_sb[:, ct, t:t + 1],
                                    None, op0=Alu.mult)
                            nc.vector.tensor_tensor(
                                out=a3[:, h0:h1, w0:w1],
                                in0=a3[:, h0:h1, w0:w1],
                                in1=p_rect, op=Alu.add)

            def pw_qk(dst_sb, xdw_sb, wT, bias, c):
                """Pointwise conv chunk -> dst[:, dblk, chunk] in [D,N] layout."""
                for dblk in range(2):
                    ps = ps_pool.tile([P, CW], f32, tag="s", name="pw_ps", bufs=3)
                    for ct in range(CT):
                        nc.tensor.matmul(
                            ps,
                            lhsT=wT[:, ct, dblk * P:(dblk + 1) * P],
                            rhs=xdw_sb[:, ct, c * CW:(c + 1) * CW],
                            start=(ct == 0), stop=(ct == CT - 1))
                    nc.scalar.activation(
                        out=dst_sb[:, dblk, c * CW:(c + 1) * CW], in_=ps,
                        func=Act.Identity, bias=bias[:, dblk:dblk + 1], scale=1.0)

            def s_tile(c, mt):
                ps = ps_pool.tile([P, CW], f32, tag="s", name="s_ps", bufs=3)
                for dblk in range(2):
                    nc.tensor.matmul(
                        ps,
                        lhsT=K_sb[:, dblk, mt * P:(mt + 1) * P],
                        rhs=Q_sb[:, dblk, c * CW:(c + 1) * CW],
                        start=(dblk == 0), stop=(dblk == 1))
                nc.scalar.activation(
                    out=pT_all[:, c, mt, :], in_=ps,
                    func=Act.Exp, scale=0.0625)

            def s_chunk(c):
                for mt in range(NT):
                    s_tile(c, mt)

            def pv_chunk(c):
                """PV + denominator + normalize + store for chunk c."""
                pv = [ps_pool.tile([P, CW], f32, tag="pv0", name="pv0"),
                      ps_pool.tile([P, CW], f32, tag="pv1", name="pv1")]
                den = ps_pool.tile([P, CW], f32, tag="den", name="den", bufs=1)
                for mt in range(NT):
                    pt = pT_all[:, c, mt, :]
                    for dblk in range(2):
                        nc.tensor.matmul(
                            pv[dblk],
                            lhsT=V_sb[:, mt, dblk * P:(dblk + 1) * P],
                            rhs=pt, start=(mt == 0), stop=(mt == NT - 1))
                    nc.tensor.matmul(den, lhsT=ones128, rhs=pt,
                                     start=(mt == 0), stop=(mt == NT - 1))
                inv = ost.tile([P, CW], f32, tag="inv", name="inv")
                nc.vector.reciprocal(inv, den)
                for dblk in range(2):
                    osb = ost.tile([P, CW], f32, tag="osb", name="osb")
                    nc.vector.scalar_tensor_tensor(
                        out=osb, in0=pv[dblk], scalar=0.0, in1=inv,
                        op0=Alu.max, op1=Alu.mult)
                    nc.sync.dma_start(
                        out_ap[dblk * P:(dblk + 1) * P, c * CW:(c + 1) * CW], osb)

            # ---- K as a folded full 3x3 conv entirely on TensorE: gives PE
            # ---- dense independent work while VectorE grinds dw(Q)/dw(V) ----
            for c in range(NCHUNK):
                r0 = c * RPC
                for dblk in range(2):
                    ps = ps_pool.tile([P, CW], f32, tag="s", name="k_ps", bufs=3)
                    first = True
                    for ct in range(CT):
                        for kh in range(3):
                            for kw in range(3):
                                nc.tensor.matmul(
                                    ps,
                                    lhsT=wsb["kw2"][:, ct, kh * 3 + kw,
                                                    dblk * P:(dblk + 1) * P],
                                    rhs=y_pad[:, ct, kh + r0:kh + r0 + RPC,
                                              kw:kw + W],
                                    start=first,
                                    stop=(ct == CT - 1 and kh == 2 and kw == 2))
                                first = False
                    nc.scalar.activation(
                        out=K_sb[:, dblk, c * CW:(c + 1) * CW], in_=ps,
                        func=Act.Identity, bias=wsb["kpb2"][:, dblk:dblk + 1],
                        scale=1.0 / 64.0)

            # ---- first query chunk on VectorE meanwhile ----
            dw_rows(xq_sb, x_sb, wsb["qdw"], wsb["qdb"], 0, RPC)
            pw_qk(Q_sb, xq_sb, wsb["qpwT"], wsb["qpb"], 0)

            # ---- V = pw(dw(x)) in [N, D] layout; its ScalarE products run
            # ---- before any exp is queued on ScalarE ----
            dw_rows(xv_sb, x_sb, wsb["vdw"], wsb["vdb"], 0, H, use_act=True)
            for nt in range(NT):
                ps = ps_pool.tile([P, D], f32, tag="s", name="v_ps", bufs=3)
                for ct in range(CT):
                    nc.tensor.matmul(
                        ps,
                        lhsT=xv_sb[:, ct, nt * P:(nt + 1) * P],
                        rhs=wsb["vpwT"][:, ct, :],
                        start=(ct == 0), stop=False)
                nc.tensor.matmul(ps, lhsT=ones1, rhs=wsb["vpb"],
                                 start=False, stop=True)
                nc.scalar.activation(out=V_sb[:, nt, :], in_=ps, func=Act.Copy)

            s_chunk(0)

            # ---- remaining chunks: S[c] interleaved with PV[c-1] ----
            for c in range(1, NCHUNK):
                dw_rows(xq_sb, x_sb, wsb["qdw"], wsb["qdb"], c * RPC, (c + 1) * RPC)
                pw_qk(Q_sb, xq_sb, wsb["qpwT"], wsb["qpb"], c)
                s_chunk(c)
                pv_chunk(c - 1)
            pv_chunk(NCHUNK - 1)

    nc.compile()
    return nc


_NC = None


def _get_nc():
    global _NC
    if _NC is None:
        _NC = _build_nc()
    return _NC


def _prep_in_maps(inputs):
    """Host-side prep: dtype casts + weight layout transforms."""
    def ctile(a):  # [C, ...] -> [P, CT, ...]
        return np.ascontiguousarray(
            a.reshape(CT, P, *a.shape[1:]).transpose(1, 0, *range(2, a.ndim + 1)))

    shared = {}
    for br, dwn, dbn, pwn, pbn in (
            ("q", "qd_w", "qd_b", "qp_w", "qp_b"),
            ("v", "vd_w", "vd_b", "vp_w", "vp_b")):
        shared[br + "dw"] = ctile(inputs[dwn].reshape(C, 9).astype(np.float32))
        shared[br + "db"] = ctile(inputs[dbn].astype(np.float32).reshape(C, 1))[:, :, 0]
        wT = inputs[pwn][:, :, 0, 0].T.astype(np.float16)  # [C, D]
        shared[br + "pwT"] = ctile(wT)
    shared["qpb"] = np.ascontiguousarray(
        inputs["qp_b"].reshape(2, P).T.astype(np.float32))
    shared["vpb"] = inputs["vp_b"].reshape(1, D).astype(np.float16)

    # folded K conv: W2[c, tap, d] = 64 * kd_w[c, tap] * kp_w[d, c]
    kd = inputs["kd_w"].reshape(C, 9).astype(np.float32)        # [C, 9]
    kp = inputs["kp_w"][:, :, 0, 0].astype(np.float32)          # [D, C]
    w2 = 64.0 * kd[:, :, None] * kp.T[:, None, :]               # [C, 9, D]
    shared["kw2"] = ctile(w2.astype(np.float16))                # [P, CT, 9, D]
    kpb2 = inputs["kp_b"].astype(np.float32) + kp @ inputs["kd_b"].astype(np.float32)
    shared["kpb2"] = np.ascontiguousarray(kpb2.reshape(2, P).T)

    x16 = inputs["x"].reshape(B, C, N).astype(np.float16)
    y16 = inputs["y"].reshape(B, C, N).astype(np.float16)
    in_maps = []
    for b in range(B):
        m = dict(shared)
        m["x"] = x16[b]
        m["y"] = y16[b]
        in_maps.append(m)
    return in_maps


def _run(inputs, **kw):
    nc = _get_nc()
    in_maps = _prep_in_maps(inputs)
    res = bass_utils.run_bass_kernel_spmd(nc, in_maps, core_ids=list(range(B)), **kw)
    return res


def kernel(**inputs):
    res = _run(inputs)
    out = np.stack([res.results[b]["out"] for b in range(B)])
    return out.reshape(B, D, H, W).astype(np.float32)
